# revision 1
# baseline (speedup 1.0000x reference)
"""Trainium2 Bass kernel for nn_MemoryEfficientBSpline (linear B-spline / KAN layer).

Math: out[b,o,p] = sum_i sum_g Wt[b,i,p,g] * coef[b,o,i,g] where Wt is the
two-hot linear-interpolation weight of x[b,i,p] over a 6-knot grid on [-1,1].

Reformulation (hinge basis): with xc = clip(x,-1,1) and nx = 2.5*(xc+1) in [0,5],
the piecewise-linear spline f(nx) = sum_k beta_k * relu(nx - k) + c0 becomes

  out[b,o,p] = alpha[b,o] + sum_i W0[b,o,i]*xc[b,i,p]
             + sum_{k=1..4} sum_i Wk[b,o,i]*relu(xc[b,i,p] + 1 - 0.4k)

i.e. 5 dense [64x64] matmuls over cheap elementwise "hinge planes" of x.
Every device plane is clip(x, lo_k, 1) with the constant shift folded into the
output bias host-side, so all 5 planes are a single DVE tensor_scalar each.

Sharding: data-parallel over batch B=8, one batch per NeuronCore. Per core the
64x36864 pixel plane is folded to 128 partitions (two pixel-halves stacked), and
the 64x64 weights are block-diagonal-duplicated to 128x128 so one full-array
matmul handles both halves.

Dtype: fp16 end-to-end on device — x, hinge planes, weights, and y are all
fp16 (PSUM accumulates fp32; bias add + downcast happen on the ACT evict).
This halves HBM traffic vs fp32 (the old bottleneck) and lets the DVE run its
4x 2-byte perf mode, leaving the PE matmuls (1 cycle/column, same rate as
fp32r) as the bottleneck. fp16's 10 mantissa bits keep rel err ~1e-3.
"""
import numpy as np
from contextlib import ExitStack

import concourse.bass as bass
import concourse.tile as tile
from concourse import bacc, mybir
from concourse.bass_utils import run_bass_kernel_spmd

# Problem shapes (hardcoded per contract)
B, IN_DIM, H, W = 8, 64, 192, 192
OUT_DIM = 64
G = 6
P_TOT = H * W          # 36864 pixels
HALF = P_TOT // 2      # 18432 (folded columns)
NPART = 128
SLAB = 2048            # columns processed per pipeline iteration
N_SLABS = HALF // SLAB # 9
CHUNK = 256            # matmul moving-operand size. Half a PSUM bank: short
                       # matmuls pipeline measurably faster through the PE
                       # than bank-sized N=512 ones (drain/fill overlap).
N_CHUNKS = SLAB // CHUNK
EVICT = 512            # ACT evict granularity (= one PSUM bank of fp32)
N_EVICTS = SLAB // EVICT
NK = 5                 # planes: clip(x, lo_k, 1) for lo = -1, -.6, -.2, .2, .6
LOS = (-1.0, -0.6, -0.2, 0.2, 0.6)

_f32 = mybir.dt.float32
_f16 = mybir.dt.float16
_Alu = mybir.AluOpType
_Act = mybir.ActivationFunctionType

_cached = None  # compiled Bass module, built once per process


def _build_module(n_reps=1):
    """n_reps>1 wraps the whole body in a hardware loop — used only for
    slope-based HW timing (dispatch noise >> exec time in this env)."""
    nc = bacc.Bacc("TRN2", target_bir_lowering=False, debug=False,
                   enable_asserts=False, num_devices=8)

    x_t = nc.dram_tensor("x", (NPART, HALF), _f16, kind="ExternalInput")
    w_t = nc.dram_tensor("wts", (NK, NPART, NPART), _f16, kind="ExternalInput")
    b_t = nc.dram_tensor("bias", (NPART, 1), _f32, kind="ExternalInput")
    y_t = nc.dram_tensor("y", (NPART, HALF), _f16, kind="ExternalOutput")

    with tile.TileContext(nc) as tc, ExitStack() as ctx:
        cpool = ctx.enter_context(tc.tile_pool(name="const", bufs=1))
        xpool = ctx.enter_context(tc.tile_pool(name="xin", bufs=4))
        ppool = ctx.enter_context(tc.tile_pool(name="planes", bufs=3))
        opool = ctx.enter_context(tc.tile_pool(name="oslab", bufs=3))
        psum = ctx.enter_context(tc.tile_pool(name="acc", bufs=2, space="PSUM"))

        # Constants: weights [128, 5*128] (block-diag per plane), bias
        wts = cpool.tile([NPART, NK * NPART], _f16)
        for k in range(NK):
            nc.sync.dma_start(wts[:, k*NPART:(k+1)*NPART], w_t[k])
        bias = cpool.tile([NPART, 1], _f32)
        nc.sync.dma_start(bias[:], b_t[:])

        def body():
            for s in range(N_SLABS):
                col0 = s * SLAB
                # Slab 0 is loaded and plane-computed in 512-col quarters so
                # the first matmul's inputs are ready sooner (pipeline fill).
                pieces = 4 if s == 0 else 1
                pw = SLAB // pieces
                xt = xpool.tile([NPART, SLAB], _f16)
                planes = [ppool.tile([NPART, SLAB], _f16, tag=f"p{k}",
                                     name=f"p{k}")
                          for k in range(NK)]
                for q in range(pieces):
                    sl = slice(q * pw, (q + 1) * pw)
                    nc.sync.dma_start(xt[:, sl], x_t[:, col0 + q*pw:col0 + (q+1)*pw])
                    for k in range(NK):
                        nc.vector.tensor_scalar(planes[k][:, sl], xt[:, sl],
                                                1.0, LOS[k], _Alu.min, _Alu.max)

                # Matmuls: 5 planes x 4 chunks accumulate into 4-bank psum
                # start=True clears has_written for the WHOLE psum bank, so
                # only the first chunk of each bank may carry it; the bank's
                # other chunks' first writes land on cleared bits (overwrite
                # semantics) which is exactly a fresh write.
                bank_chunks = EVICT // CHUNK
                acc = psum.tile([NPART, SLAB], _f32)
                for k in range(NK):
                    wk = wts[:, k*NPART:(k+1)*NPART]
                    for c in range(N_CHUNKS):
                        nc.tensor.matmul(acc[:, c*CHUNK:(c+1)*CHUNK], wk,
                                         planes[k][:, c*CHUNK:(c+1)*CHUNK],
                                         start=(k == 0 and c % bank_chunks == 0),
                                         stop=(k == NK - 1),
                                         skip_group_check=True)

                # Evict + bias on ACT per psum bank, DMA out per half slab.
                # Finer granularity frees psum banks (and the next slab's
                # matmuls) sooner and shrinks the end-of-stream drain tail.
                ot = opool.tile([NPART, SLAB], _f16)
                for c in range(N_EVICTS):
                    cs = slice(c*EVICT, (c+1)*EVICT)
                    nc.scalar.activation(ot[:, cs], acc[:, cs], _Act.Identity,
                                         bias=bias[:], scale=1.0)
                    if c % 2 == 1:
                        hs = slice((c-1)*EVICT, (c+1)*EVICT)
                        nc.sync.dma_start(y_t[:, col0+(c-1)*EVICT:col0+(c+1)*EVICT],
                                          ot[:, hs])

        for _ in range(n_reps):
            body()

    if DEDUP_LDW:
        _dedup_ldweights(nc)
    nc.compile()
    return nc


DEDUP_LDW = False  # measured 4us SLOWER: per-MM LDWs feed bg weight-buffer pipelining


def _dedup_ldweights(nc):
    """Drop InstLdweights whose weights AP matches the immediately preceding
    LDW in the same block: the PE array already holds those weights, and for
    2-byte dtypes a matmul may run non-self-loading. Cuts 8x redundant weight
    reloads per plane pass (tile emits one LDW per matmul unconditionally)."""
    for b in nc.m.functions[0].blocks:
        insts = b.instructions
        keep = []
        prev_sig = None
        for i in insts:
            if type(i).__name__ == 'InstLdweights':
                sig = str(i.ins[0])
                if sig == prev_sig:
                    continue
                prev_sig = sig
            keep.append(i)
        if len(keep) != len(insts):
            insts[:] = keep


def _get_module():
    global _cached
    if _cached is None:
        _cached = _build_module()
    return _cached


def _prep_inputs(x, coef):
    """Host-side shard + coefficient transform. Returns in_maps for 8 cores."""
    x = np.asarray(x, dtype=np.float32)
    c = np.asarray(coef, dtype=np.float64)            # [B, o, i, 6]
    d = np.diff(c, axis=-1)                           # [B, o, i, 5]
    beta = np.concatenate([d[..., :1], np.diff(d, axis=-1)], axis=-1)
    Wk = (2.5 * beta).astype(np.float16)              # [B, o, i, 5]
    Wk64 = Wk.astype(np.float64)
    # Device planes are clip(x, lo_k, 1) = relu(xc + c_k) - c_k with
    # c_k = 1 - 0.4k = -lo_k: fold the +c_k shift into the bias.
    alpha = (c[..., 0].sum(axis=2) + Wk64[..., 0].sum(axis=2)
             + sum((1.0 - 0.4 * k) * Wk64[..., k].sum(axis=2) for k in (1, 2, 3, 4))
             ).astype(np.float32)                     # [B, o]

    in_maps = []
    eye2 = np.eye(2, dtype=np.float16)
    for b in range(B):
        xb = x[b].reshape(IN_DIM, P_TOT)
        x_f = np.concatenate([xb[:, :HALF], xb[:, HALF:]], axis=0)  # [128, HALF]
        # lhsT[k][i, o] = Wk[b, o, i, k], block-diag duplicated to 128x128
        lhsT = np.einsum('oik->kio', Wk[b])           # [5, i, o]
        wts = np.kron(eye2, lhsT).astype(np.float16)  # [5, 128, 128]
        bias = np.tile(alpha[b], 2).reshape(NPART, 1).astype(np.float32)
        in_maps.append({
            "x": np.ascontiguousarray(x_f.astype(np.float16)),
            "wts": np.ascontiguousarray(wts),
            "bias": bias,
        })
    return in_maps


def _assemble(results):
    out = np.empty((B, OUT_DIM, H, W), dtype=np.float32)
    for b in range(B):
        y_f = results[b]["y"].astype(np.float32)       # [128, HALF]
        out[b] = np.concatenate([y_f[:OUT_DIM], y_f[OUT_DIM:]], axis=1).reshape(OUT_DIM, H, W)
    return out


def run(x, coef, **spmd_kwargs):
    """Run on 8 NeuronCores; returns (output, BassKernelResults)."""
    nc = _get_module()
    in_maps = _prep_inputs(x, coef)
    res = run_bass_kernel_spmd(nc, in_maps, core_ids=list(range(8)), **spmd_kwargs)
    return _assemble(res.results), res


def kernel(x, coef):
    out, _ = run(x, coef)
    return out



# revision 18
# speedup vs baseline: 1.2962x; 1.2962x over previous
"""Trainium2 Bass kernel for nn_MemoryEfficientBSpline — 2x2 PE-tiled version.

Math (hinge basis, see baseline docstring): per core (one batch b),
  out[o,p] = alpha[o] + sum_{k=0..4} sum_i Wk[o,i] * plane_k[i,p]
with plane_k = clip(x, lo_k, 1), lo = (-1,-.6,-.2,.2,.6), weights/bias
precomputed host-side from coef.

PE scheme: the 128x128 array is addressed as a 2x2 grid of 64x64 tiles
(tile_position row/col groups in {0,64}). The 4 tiles run CONCURRENT
matmul streams (HW-measured 0.156 ns/tile-col for this pattern vs 0.42
for the old block-diag full-array fold at 2 pixel-cols/cycle — ~1.35x
net). Each tile owns its own (psum partition-slice, psum bank) slot so
the 5-plane accumulation stays tile-local (two tiles sharing a column
group MUST use different psum banks — same-bank fails in hardware):

  per 1024-col round r2 of a 2048-col slab (folded x: top half pixels on
  partitions 0:64, bottom on 64:128):
    tile(0,0):   psum[0:64,   base+0:512]   <- planes[0:64,   base+0:512]
    tile(64,0):  psum[0:64,   base+512:1024]<- planes[64:128, base+0:512]
    tile(0,64):  psum[64:128, base+0:512]   <- planes[0:64,   base+512:1024]
    tile(64,64): psum[64:128, base+512:1024]<- planes[64:128, base+512:1024]

so psum/y columns hold a block-scrambled pixel layout that the host
unscrambles for free in _assemble.

Weights rotate through each tile per plane k (k-outer loop, both rounds of
a slab per load); redundant per-round LDWEIGHTS are deduped per tile
position. Evictions (ACT, +bias, ->fp16) go per 512-col psum bank; DMA
issue is split between the SP HWDGE queue (y stores + first x pieces) and
the Pool/gpsimd SWDGE queue (steady x loads) to keep descriptor-gen off
the critical path. A PE warmup burst (zero matmuls into the first psum
tile during the DMA fill) ramps the tensor-engine p-state before real
work arrives, and a dummy activation preloads the ACT function table.
"""
import numpy as np
from contextlib import ExitStack

import concourse.bass as bass
import concourse.tile as tile
from concourse import bacc, mybir
from concourse.bass_utils import run_bass_kernel_spmd

B, IN_DIM, H, W = 8, 64, 192, 192
OUT_DIM = 64
G = 6
P_TOT = H * W          # 36864
HALF = P_TOT // 2      # 18432 folded columns
NPART = 128
SLAB = 2048
N_SLABS = HALF // SLAB # 9
NK = 5
LOS = (-1.0, -0.6, -0.2, 0.2, 0.6)
N_WARM = 20            # PE p-state warmup matmuls (128 cols each)

_f32 = mybir.dt.float32
_f16 = mybir.dt.float16
_Alu = mybir.AluOpType
_Act = mybir.ActivationFunctionType

# (row_grp, col_grp, moving col offset, psum col offset) per tile
TILES = ((0, 0, 0, 0), (64, 0, 0, 512), (0, 64, 512, 0), (64, 64, 512, 512))

_cached = None


def _dedup_ldweights(nc):
    """Drop InstLdweights whose (tile_position, weights AP) matches the last
    LDW at the same position — the sub-array already holds those weights."""
    for blk in nc.m.functions[0].blocks:
        insts = blk.instructions
        keep = []
        last = {}
        for i in insts:
            tn = type(i).__name__
            if tn == 'InstLdweights':
                pos = tuple(i.tile_position) if i.tile_position else (0, 0)
                sig = str(i.ins[0])
                if last.get(pos) == sig:
                    continue
                last[pos] = sig
            elif tn == 'InstMatmult' and (i.tile_position is None
                                          or tuple(i.tile_position) == (0, 0)):
                # full-array matmul (warmup) clobbers every sub-array
                last.clear()
            keep.append(i)
        if len(keep) != len(insts):
            insts[:] = keep


DEFAULT_CFG = dict(
    slab=2048,          # folded cols per x/plane tile (multiple of 1024)
    psum_round=False,   # psum tile per 1024-col round (bufs=4) vs per slab (bufs=2)
    y_per_slab=False,   # one y DMA per slab vs per 1024-col round
    x_queue="pool",     # steady x loads: "pool" (SWDGE) or "sync" (HWDGE)
    y_queue="sync",
    order="k_outer",    # k_outer: LDW amortized over rounds; r2_outer: evicts spread
    palindrome=False,   # alternate k direction per slab (neutral; off with row_phase)
    row_phase=True,     # desync row groups: top tiles on plane k while bottom
                        # tiles run it next, so each row's LDW hides under the
                        # other row's matmuls
    evict_dve=False,    # offload one bank's evict per slab to DVE
    n_warm=N_WARM,
    dedup=True,
)


def _build_module(n_reps=1, cfg=None):
    c = dict(DEFAULT_CFG)
    if cfg:
        c.update(cfg)
    slab = c["slab"]
    n_slabs = HALF // slab
    n_rounds = slab // 1024
    assert HALF % slab == 0 and slab % 1024 == 0

    nc = bacc.Bacc("TRN2", target_bir_lowering=False, debug=False,
                   enable_asserts=False, num_devices=8)

    x_t = nc.dram_tensor("x", (NPART, HALF), _f16, kind="ExternalInput")
    w_t = nc.dram_tensor("wts", (NPART, NK * 64), _f16, kind="ExternalInput")
    b_t = nc.dram_tensor("bias", (NPART, 1), _f32, kind="ExternalInput")
    y_t = nc.dram_tensor("y", (NPART, HALF), _f16, kind="ExternalOutput")

    xeng = {"pool": nc.gpsimd, "sync": nc.sync}[c["x_queue"]]
    yeng = {"pool": nc.gpsimd, "sync": nc.sync}[c["y_queue"]]

    with tile.TileContext(nc) as tc, ExitStack() as ctx:
        cpool = ctx.enter_context(tc.tile_pool(name="const", bufs=1))
        xpool = ctx.enter_context(tc.tile_pool(name="xin", bufs=4))
        ppool = ctx.enter_context(tc.tile_pool(name="planes", bufs=3))
        opool = ctx.enter_context(tc.tile_pool(name="oslab", bufs=3))
        if c["psum_round"]:
            psum = ctx.enter_context(tc.tile_pool(name="acc", bufs=4, space="PSUM"))
        else:
            assert slab == 2048
            psum = ctx.enter_context(tc.tile_pool(name="acc", bufs=2, space="PSUM"))

        wts = cpool.tile([NPART, NK * 64], _f16)
        bias = cpool.tile([NPART, 1], _f32)
        warm = cpool.tile([NPART, NPART], _f16)
        awm = cpool.tile([NPART, 1], _f16)

        # prologue (once, outside the rep loop)
        nc.vector.memset(warm[:], 0.25)
        nc.sync.dma_start(wts[:], w_t[:])            # HWDGE: critical path
        nc.scalar.activation(awm[:], warm[:, :1], _Act.Identity, scale=1.0)

        def body(first):
            for s in range(n_slabs):
                col0 = s * slab
                xt = xpool.tile([NPART, slab], _f16)
                if first and s == 0:
                    # two pieces on separate DGE paths
                    h0 = slab // 2
                    nc.sync.dma_start(xt[:, :h0], x_t[:, col0:col0 + h0])
                    nc.gpsimd.dma_start(xt[:, h0:], x_t[:, col0 + h0:col0 + slab])
                    nc.sync.dma_start(bias[:], b_t[:])
                else:
                    xeng.dma_start(xt[:], x_t[:, col0:col0 + slab])

                # x arrives pre-clipped to [-1, 1] (host), so plane 0 is xt
                # itself and planes 1-4 are a single max against lo_k.
                planes = [None] + [ppool.tile([NPART, slab], _f16, tag=f"p{k}",
                                              name=f"p{k}") for k in range(1, NK)]
                if first and s == 0:
                    for q in (0, 1):
                        sl = slice(q * (slab // 2), (q + 1) * (slab // 2))
                        for k in range(1, NK):
                            nc.vector.tensor_scalar(planes[k][:, sl], xt[:, sl],
                                                    LOS[k], None, _Alu.max)
                else:
                    for k in range(1, NK):
                        nc.vector.tensor_scalar(planes[k][:], xt[:],
                                                LOS[k], None, _Alu.max)

                last_slab = s == n_slabs - 1
                if c["psum_round"]:
                    accs = [psum.tile([NPART, 1024], _f32, name=f"acc{r}")
                            for r in range(n_rounds)]
                else:
                    acc_slab = psum.tile([NPART, slab], _f32)
                ot = opool.tile([NPART, slab], _f16)

                def acc_ap(r2, pstart, cols):
                    # psum AP for round r2, partitions pstart:+64, cols within round
                    if c["psum_round"]:
                        return accs[r2][pstart:pstart + 64, cols]
                    return acc_slab[pstart:pstart + 64,
                                    slice(1024 * r2 + cols.start, 1024 * r2 + cols.stop)]

                if first and s == 0:
                    # PE p-state warmup during the DMA fill; overwritten by
                    # the real bank-0 accumulation below (start=True clears).
                    wacc = accs[0] if c["psum_round"] else acc_slab
                    for _ in range(c["n_warm"]):
                        nc.tensor.matmul(wacc[:, :NPART], warm[:], warm[:],
                                         start=True, stop=True,
                                         skip_group_check=True)

                ks = list(range(NK))
                if c["palindrome"] and s % 2 == 1:
                    ks.reverse()

                def mm1(k, r2, tiles):
                    base = 1024 * r2
                    src = xt if k == 0 else planes[k]
                    for (tr, tc2, mv, pc) in tiles:
                        nc.tensor.matmul(
                            acc_ap(r2, tc2, slice(pc, pc + 512)),
                            wts[tr:tr + 64, k * 64:(k + 1) * 64],
                            src[tr:tr + 64, base + mv:base + mv + 512],
                            start=(k == ks[0]), stop=(k == ks[-1]),
                            skip_group_check=True,
                            tile_position=(tr, tc2))

                def mm(k, r2):
                    mm1(k, r2, TILES)

                def evict(r2, last_round):
                    base = 1024 * r2
                    for hb in (0, 1):
                        cols = slice(hb * 512, (hb + 1) * 512)
                        bs = slice(base + hb * 512, base + (hb + 1) * 512)
                        src = (accs[r2][:, cols] if c["psum_round"]
                               else acc_slab[:, bs])
                        if c["evict_dve"] and last_round and hb == 1:
                            # offload one bank to DVE (measured SLOWER; off)
                            nc.vector.tensor_scalar(ot[:, bs], src,
                                                    bias[:], None, _Alu.add)
                        else:
                            nc.scalar.activation(ot[:, bs], src,
                                                 _Act.Identity, bias=bias[:],
                                                 scale=1.0)
                    if last_slab and last_round:
                        for hb in (0, 1):
                            bs = slice(base + hb * 512, base + (hb + 1) * 512)
                            eng = nc.gpsimd if hb == 0 else nc.sync
                            eng.dma_start(y_t[:, col0 + base + hb * 512:
                                               col0 + base + (hb + 1) * 512],
                                          ot[:, bs])
                    elif not c["y_per_slab"]:
                        yeng.dma_start(y_t[:, col0 + base:col0 + base + 1024],
                                       ot[:, base:base + 1024])

                if (first and s == 0) or c["order"] == "r2_outer":
                    # round-outer: first slab's second x piece arrives late;
                    # also spreads evicts through the matmul stream
                    for r2 in range(n_rounds):
                        for k in ks:
                            mm(k, r2)
                        evict(r2, r2 == n_rounds - 1)
                elif c["row_phase"]:
                    # row-group software pipeline: per plane, top tiles first,
                    # then bottom — each row's next-plane LDW overlaps the
                    # other row's matmuls.
                    top = [t for t in TILES if t[0] == 0]
                    bot = [t for t in TILES if t[0] == 64]
                    for k in ks:
                        for r2 in range(n_rounds):
                            mm1(k, r2, top)
                        for r2 in range(n_rounds):
                            mm1(k, r2, bot)
                    for r2 in range(n_rounds):
                        evict(r2, r2 == n_rounds - 1)
                else:
                    # k-outer: one weight rotation serves all rounds
                    for k in ks:
                        for r2 in range(n_rounds):
                            mm(k, r2)
                    for r2 in range(n_rounds):
                        evict(r2, r2 == n_rounds - 1)
                if c["y_per_slab"] and not last_slab:
                    yeng.dma_start(y_t[:, col0:col0 + slab], ot[:])

        for rep in range(n_reps):
            body(rep == 0)

    if c["dedup"]:
        _dedup_ldweights(nc)
    nc.compile()
    return nc


def _get_module():
    global _cached
    if _cached is None:
        _cached = _build_module()
    return _cached


def _prep_inputs(x, coef):
    """Host-side shard + coefficient transform. Returns in_maps for 8 cores."""
    x = np.asarray(x, dtype=np.float32)
    c = np.asarray(coef, dtype=np.float64)            # [B, o, i, 6]
    d = np.diff(c, axis=-1)                           # [B, o, i, 5]
    beta = np.concatenate([d[..., :1], np.diff(d, axis=-1)], axis=-1)
    Wk = (2.5 * beta).astype(np.float16)              # [B, o, i, 5]
    Wk64 = Wk.astype(np.float64)
    # plane_k = clip(x, lo_k, 1) = relu(xc + c_k) - c_k with c_k = -lo_k:
    # fold the constant shift into the bias.
    alpha = (c[..., 0].sum(axis=2) + Wk64[..., 0].sum(axis=2)
             + sum((1.0 - 0.4 * k) * Wk64[..., k].sum(axis=2) for k in (1, 2, 3, 4))
             ).astype(np.float32)                     # [B, o]

    in_maps = []
    x = np.clip(x, -1.0, 1.0)   # device plane 0 is then x itself
    for b in range(B):
        xb = x[b].reshape(IN_DIM, P_TOT)
        x_f = np.concatenate([xb[:, :HALF], xb[:, HALF:]], axis=0)  # [128, HALF]
        lhsT = np.einsum('oik->kio', Wk[b])           # [5, i, o]
        wts = np.concatenate([lhsT[k] for k in range(NK)], axis=1)  # [64, 320]
        wts = np.concatenate([wts, wts], axis=0)      # [128, 320] both row grps
        bias = np.tile(alpha[b], 2).reshape(NPART, 1).astype(np.float32)
        in_maps.append({
            "x": np.ascontiguousarray(x_f.astype(np.float16)),
            "wts": np.ascontiguousarray(wts.astype(np.float16)),
            "bias": bias,
        })
    return in_maps


def _assemble(results):
    out = np.empty((B, OUT_DIM, H, W), dtype=np.float32)
    n_r = HALF // 1024                                 # 18 rounds
    for b in range(B):
        y = results[b]["y"].astype(np.float32)         # [128, HALF] scrambled
        yr = y.reshape(2, OUT_DIM, n_r, 2, 512)        # (p2, o, r, h, c)
        # folded col of (p2=j, r, h, c) block = 1024r + 512j + c; h selects
        # top (0) / bottom (1) pixel half.
        top = yr[:, :, :, 0, :].transpose(1, 2, 0, 3).reshape(OUT_DIM, HALF)
        bot = yr[:, :, :, 1, :].transpose(1, 2, 0, 3).reshape(OUT_DIM, HALF)
        out[b] = np.concatenate([top, bot], axis=1).reshape(OUT_DIM, H, W)
    return out


def run(x, coef, **spmd_kwargs):
    nc = _get_module()
    in_maps = _prep_inputs(x, coef)
    res = run_bass_kernel_spmd(nc, in_maps, core_ids=list(range(8)), **spmd_kwargs)
    return _assemble(res.results), res


def kernel(x, coef):
    out, _ = run(x, coef)
    return out
